# revision 13
# baseline (speedup 1.0000x reference)
"""Trainium2 Bass kernel for nn_AttentionBlock (GroupNorm + 4-head self-attention
+ projection + residual) on x:(16,512,32,32) fp32.

Sharding: data-parallel over batch across 8 NeuronCores (2 batch elements per
core), weights replicated, no collectives.

Per-core dataflow (channels-on-partitions layouts):
  GroupNorm: bn_stats per channel -> group reduce/broadcast via tiny selector
             matmuls -> xn = x*A + B (fp16)
  QKV: q,k in (o,s) layout, v computed transposed (s,o) by using xn as the
       stationary matmul operand.
  Attention per head, transposed layout, no max subtraction (|S| < ~8 so exp
       is safe): ST = k^T q -> E = exp(ST) (fp16, ScalarE) -> denominator
       broadcast = ones^T @ (pairwise-summed E) -> fast reciprocal (VectorE)
       -> out = (vT^T @ E) * recip
  Proj: bias + residual fused into one scalar_tensor_tensor (fp32, exact
       residual).

All matmuls are fp16 (11-bit mantissa; accumulation is always fp32 in PSUM).
This is deliberate: fp16 matmuls lower to LDWEIGHTS+MATMUL which together
carry two sync waits, while f32/f32r self-loading matmuls carry only ONE --
walrus rejects multi-source dependency patterns otherwise. All constants
arrive in a single fp16 DMA (f32 biases bit-packed into fp16 pairs), x in one
DMA, outputs in one DMA: 3 DMA queues + 3 engines keeps the kernel-tail drain
under the walrus sync-wait cap.
"""

import numpy as np

B, C, H, W = 16, 512, 32, 32
N = H * W  # 1024 spatial positions
HEADS = 4
HD = C // HEADS  # 128
EPS = 1e-5
NCORES = 8
BPC = B // NCORES  # 2 batches per core
CT = C // 128  # 4 channel tiles
NT = N // 512  # 2 free-dim tiles of 512

# fp16 constant-pack column offsets
_HOFF_WQKV = 0  # (128, CT*3C) = 6144
_HOFF_WPROJ = _HOFF_WQKV + CT * 3 * C  # (128, CT*C) = 2048
_HOFF_BV = _HOFF_WPROJ + CT * C  # row 0: (1, C)
_HOFF_ONES = _HOFF_BV + C  # (128, 128) of 1.0
_HOFF_SEL = _HOFF_ONES + 128  # (128, 8) group selector
_HOFF_SELT = _HOFF_SEL + 8  # rows 0:8: (8, 128)
_HOFF_F32 = _HOFF_SELT + 128  # f32 consts as fp16 hi+lo pairs: 20 f32 = 2x20
_PACK16_W = _HOFF_F32 + 40

_CACHE = {}


def build_program():
    import concourse.bass as bass
    import concourse.tile as tile
    from concourse import bacc, mybir

    f32 = mybir.dt.float32
    fp16 = mybir.dt.float16
    FT = mybir.ActivationFunctionType
    OP = mybir.AluOpType

    nc = bacc.Bacc(None, target_bir_lowering=False)

    x_d = nc.dram_tensor("x", [BPC, CT, 128, N], f32, kind="ExternalInput")
    pack_d = nc.dram_tensor("pack", [128, _PACK16_W], fp16, kind="ExternalInput")
    out_d = nc.dram_tensor("out", [BPC, CT, 128, N], f32, kind="ExternalOutput")

    ctx = tile.TileContext(nc)
    with ctx as tc, tc.tile_pool(name="consts", bufs=1) as consts:
        wp = consts.tile([128, _PACK16_W], fp16, tag="wp")
        nc.sync.dma_start(out=wp, in_=pack_d[:])
        wqkvT = wp[:, _HOFF_WQKV : _HOFF_WQKV + CT * 3 * C].rearrange(
            "p (t o) -> p t o", t=CT
        )
        wprojT = wp[:, _HOFF_WPROJ : _HOFF_WPROJ + CT * C].rearrange(
            "p (t o) -> p t o", t=CT
        )
        bvrow = wp[0:1, _HOFF_BV : _HOFF_BV + C]
        ones_h = wp[:, _HOFF_ONES : _HOFF_ONES + 128]
        ones1 = ones_h[0:1, :]
        sel = wp[:, _HOFF_SEL : _HOFF_SEL + 8]
        selT = wp[0:8, _HOFF_SELT : _HOFF_SELT + 128]
        # f32 bias constants travel as fp16 hi+lo pairs (bitcasting f32 into
        # fp16 lanes can produce NaN patterns the simulator rejects)
        fbias = consts.tile([128, 20], f32, tag="fbias")
        nc.vector.tensor_add(
            out=fbias,
            in0=wp[:, _HOFF_F32 : _HOFF_F32 + 20],
            in1=wp[:, _HOFF_F32 + 20 : _HOFF_F32 + 40],
        )
        bqk = fbias[:, 0:8]
        gb = fbias[:, 8:16].rearrange("p (t two) -> p t two", t=CT)
        bproj = fbias[:, 16:20]
        eps8 = consts.tile([8, 1], f32, tag="eps8")
        nc.vector.memset(eps8, EPS)

        with (
            tc.tile_pool(name="sb", bufs=1) as sb,
            tc.tile_pool(name="ps", bufs=1, space="PSUM") as ps,
        ):
            # all of x for both local batches in one DMA
            x_all = sb.tile([128, BPC, CT, N], f32, tag="x")
            nc.sync.dma_start(out=x_all, in_=x_d.rearrange("b t p n -> p b t n"))
            out_all = sb.tile([128, BPC, CT, N], f32, tag="outsb")

            for b in range(BPC):
                x_b = x_all[:, b]

                # ---------------- GroupNorm ----------------
                gst = sb.tile([8, CT, 3], f32, tag="gst", bufs=2)
                for t in range(CT):
                    stats6 = sb.tile([128, 2, 6], f32, tag="stats6", bufs=2)
                    xv = x_b[:, t, :].rearrange("p (a c) -> p a c", c=512)
                    for a in range(2):
                        nc.vector.bn_stats(out=stats6[:, a, :], in_=xv[:, a, :])
                    stats3 = sb.tile([128, 3], fp16, tag="stats3", bufs=2)
                    nc.vector.bn_aggr(out=stats3[:, 0:2], in_=stats6)
                    nc.vector.tensor_mul(
                        out=stats3[:, 2:3], in0=stats3[:, 0:1], in1=stats3[:, 0:1]
                    )
                    gsum_ps = ps.tile([8, 3], f32, tag="work", bufs=2)
                    nc.tensor.matmul(gsum_ps, sel, stats3, start=True, stop=True)
                    nc.vector.tensor_copy(out=gst[:, t, :], in_=gsum_ps)

                # group mean/var/rstd on (8, CT) views; rstd = exp(-0.5*ln(var+eps))
                rg = sb.tile([8, CT, 2], fp16, tag="rg", bufs=2)
                nc.vector.tensor_scalar(
                    out=rg[:, :, 1],
                    in0=gst[:, :, 0],
                    scalar1=1.0 / 16.0,
                    scalar2=None,
                    op0=OP.mult,
                )
                ex2 = sb.tile([8, CT], f32, tag="ex2", bufs=2)
                nc.vector.tensor_add(out=ex2, in0=gst[:, :, 1], in1=gst[:, :, 2])
                nc.vector.tensor_scalar(
                    out=ex2, in0=ex2, scalar1=1.0 / 16.0, scalar2=None, op0=OP.mult
                )
                msq = sb.tile([8, CT], f32, tag="msq", bufs=2)
                nc.vector.tensor_mul(out=msq, in0=rg[:, :, 1], in1=rg[:, :, 1])
                nc.vector.tensor_sub(out=ex2, in0=ex2, in1=msq)
                lnv = sb.tile([8, CT], f32, tag="lnv", bufs=2)
                nc.scalar.activation(out=lnv, in_=ex2, func=FT.Ln, bias=eps8)
                rr = sb.tile([8, CT], f32, tag="rr", bufs=2)
                nc.scalar.activation(out=rr, in_=lnv, func=FT.Exp, scale=-0.5)
                nc.vector.tensor_copy(out=rg[:, :, 0], in_=rr)

                ab = sb.tile([128, CT, 2], f32, tag="ab", bufs=2)
                for t in range(CT):
                    cb_ps = ps.tile([128, 2], f32, tag="work", bufs=2)
                    nc.tensor.matmul(cb_ps, selT, rg[:, t, :], start=True, stop=True)
                    nc.vector.tensor_mul(
                        out=ab[:, t, 0:1], in0=cb_ps[:, 0:1], in1=gb[:, t, 0:1]
                    )
                    tmp1 = sb.tile([128, 1], f32, tag="tmp1", bufs=2)
                    nc.vector.tensor_mul(out=tmp1, in0=cb_ps[:, 1:2], in1=ab[:, t, 0:1])
                    nc.vector.tensor_sub(out=ab[:, t, 1:2], in0=gb[:, t, 1:2], in1=tmp1)

                xn = sb.tile([128, CT, N], fp16, tag="xn", bufs=1)
                for t in range(CT):
                    nc.vector.tensor_scalar(
                        out=xn[:, t, :],
                        in0=x_b[:, t, :],
                        scalar1=ab[:, t, 0:1],
                        scalar2=ab[:, t, 1:2],
                        op0=OP.mult,
                        op1=OP.add,
                    )

                # ---------------- QKV ----------------
                qk = sb.tile([128, 8, N], fp16, tag="qk", bufs=1)
                for m in range(8):
                    qkps = ps.tile([128, N], f32, tag="work", bufs=2)
                    for kk in range(CT):
                        lhs = wqkvT[:, kk, m * 128 : (m + 1) * 128]
                        for n in range(NT):
                            nc.tensor.matmul(
                                qkps[:, n * 512 : (n + 1) * 512],
                                lhs,
                                xn[:, kk, n * 512 : (n + 1) * 512],
                                start=(kk == 0),
                                stop=(kk == CT - 1),
                            )
                    nc.vector.tensor_scalar(
                        out=qk[:, m, :],
                        in0=qkps,
                        scalar1=bqk[:, m : m + 1],
                        scalar2=None,
                        op0=OP.add,
                    )

                vT = sb.tile([128, 8, C], fp16, tag="vT", bufs=1)
                for sm in range(8):
                    vps = ps.tile([128, C], f32, tag="work", bufs=2)
                    nc.tensor.matmul(vps, ones1, bvrow, start=True, stop=False)
                    for kk in range(CT):
                        nc.tensor.matmul(
                            vps,
                            xn[:, kk, sm * 128 : (sm + 1) * 128],
                            wqkvT[:, kk, 2 * C : 3 * C],
                            start=False,
                            stop=(kk == CT - 1),
                        )
                    nc.vector.tensor_copy(out=vT[:, sm, :], in_=vps)

                # ---------------- attention ----------------
                attn = sb.tile([128, CT, N], fp16, tag="attn", bufs=1)
                for h in range(HEADS):
                    q_h = qk[:, h, :]
                    k_h = qk[:, 4 + h, :]
                    av_ps = ps.tile([128, N], f32, tag="avdn", bufs=2)
                    dn_ps = ps.tile([128, N], f32, tag="avdn", bufs=2)
                    prev_e = None
                    for mt in range(8):
                        st_ps = ps.tile([128, N], f32, tag="work", bufs=2)
                        lhs_k = k_h[:, mt * 128 : (mt + 1) * 128]
                        for n in range(NT):
                            nc.tensor.matmul(
                                st_ps[:, n * 512 : (n + 1) * 512],
                                lhs_k,
                                q_h[:, n * 512 : (n + 1) * 512],
                                start=True,
                                stop=True,
                            )
                        e_t = sb.tile([128, N], fp16, tag="E", bufs=8)
                        nc.scalar.activation(out=e_t, in_=st_ps, func=FT.Exp)
                        lhs_v = vT[:, mt, h * 128 : (h + 1) * 128]
                        for n in range(NT):
                            nc.tensor.matmul(
                                av_ps[:, n * 512 : (n + 1) * 512],
                                lhs_v,
                                e_t[:, n * 512 : (n + 1) * 512],
                                start=(mt == 0),
                                stop=(mt == 7),
                            )
                        if mt % 2 == 0:
                            prev_e = e_t
                        else:
                            j = mt // 2
                            pr = sb.tile([128, N], fp16, tag="pair", bufs=3)
                            nc.vector.tensor_add(out=pr, in0=prev_e, in1=e_t)
                            for n in range(NT):
                                nc.tensor.matmul(
                                    dn_ps[:, n * 512 : (n + 1) * 512],
                                    ones_h,
                                    pr[:, n * 512 : (n + 1) * 512],
                                    start=(j == 0),
                                    stop=(j == 3),
                                )
                    rec = sb.tile([128, N], f32, tag="rec", bufs=2)
                    nc.vector.reciprocal_approx_fast(out=rec, in_=dn_ps)
                    nc.vector.tensor_mul(out=attn[:, h, :], in0=av_ps, in1=rec)

                # ---------------- proj + bias + residual ----------------
                for m in range(CT):
                    prps = ps.tile([128, N], f32, tag="work", bufs=2)
                    for t in range(CT):
                        lhs = wprojT[:, t, m * 128 : (m + 1) * 128]
                        for n in range(NT):
                            nc.tensor.matmul(
                                prps[:, n * 512 : (n + 1) * 512],
                                lhs,
                                attn[:, t, n * 512 : (n + 1) * 512],
                                start=(t == 0),
                                stop=(t == CT - 1),
                            )
                    nc.vector.scalar_tensor_tensor(
                        out=out_all[:, b, m, :],
                        in0=prps,
                        scalar=bproj[:, m : m + 1],
                        in1=x_b[:, m, :],
                        op0=OP.add,
                        op1=OP.add,
                    )

            nc.sync.dma_start(
                out=out_d.rearrange("b t p n -> p b t n"), in_=out_all
            )

    nc.finalize()
    return nc


def _host_prep(x, gamma, beta, w_qkv, b_qkv, w_proj, b_proj):
    scale = float(HD) ** -0.5
    x = np.asarray(x, np.float32)
    gamma = np.asarray(gamma, np.float32)
    beta = np.asarray(beta, np.float32)
    w_qkv = np.asarray(w_qkv, np.float32)
    b_qkv = np.asarray(b_qkv, np.float32)
    w_proj = np.asarray(w_proj, np.float32)
    b_proj = np.asarray(b_proj, np.float32)

    wq = w_qkv.copy()
    wq[:C] *= scale  # fold the attention scale into the q weights/bias
    bq = b_qkv.copy()
    bq[:C] *= scale

    pack = np.zeros((128, _PACK16_W), np.float16)
    pack[:, _HOFF_WQKV : _HOFF_WQKV + CT * 3 * C] = (
        np.ascontiguousarray(wq.T)
        .reshape(CT, 128, 3 * C)
        .transpose(1, 0, 2)
        .reshape(128, -1)
        .astype(np.float16)
    )
    pack[:, _HOFF_WPROJ : _HOFF_WPROJ + CT * C] = (
        np.ascontiguousarray(w_proj.T)
        .reshape(CT, 128, C)
        .transpose(1, 0, 2)
        .reshape(128, -1)
        .astype(np.float16)
    )
    pack[0, _HOFF_BV : _HOFF_BV + C] = bq[2 * C :].astype(np.float16)
    pack[:, _HOFF_ONES : _HOFF_ONES + 128] = np.float16(1.0)
    selm = np.zeros((128, 8), np.float16)
    selm[np.arange(128), np.arange(128) // 16] = np.float16(1.0)
    pack[:, _HOFF_SEL : _HOFF_SEL + 8] = selm
    pack[0:8, _HOFF_SELT : _HOFF_SELT + 128] = selm.T

    fpart = np.zeros((128, 20), np.float32)
    fpart[:, 0:8] = bq[: 2 * C].reshape(8, 128).T
    fpart[:, 8:16] = (
        np.stack([gamma.reshape(CT, 128), beta.reshape(CT, 128)], axis=-1)
        .transpose(1, 0, 2)
        .reshape(128, -1)
    )
    fpart[:, 16:20] = b_proj.reshape(CT, 128).T
    hi = fpart.astype(np.float16)
    lo = (fpart - hi.astype(np.float32)).astype(np.float16)
    pack[:, _HOFF_F32 : _HOFF_F32 + 20] = hi
    pack[:, _HOFF_F32 + 20 : _HOFF_F32 + 40] = lo

    x_r = np.ascontiguousarray(x.reshape(B, CT, 128, N))
    in_maps = [
        {
            "pack": pack,
            "x": np.ascontiguousarray(
                x_r[c * BPC : (c + 1) * BPC].reshape(BPC, CT, 128, N)
            ),
        }
        for c in range(NCORES)
    ]
    return in_maps


def kernel(x, gamma, beta, w_qkv, b_qkv, w_proj, b_proj, _trace=False):
    from concourse.bass_utils import run_bass_kernel_spmd

    if "nc" not in _CACHE:
        _CACHE["nc"] = build_program()
    nc = _CACHE["nc"]

    in_maps = _host_prep(x, gamma, beta, w_qkv, b_qkv, w_proj, b_proj)
    res = run_bass_kernel_spmd(
        nc, in_maps, core_ids=list(range(NCORES)), trace=_trace
    )
    _CACHE["last_result"] = res
    out = np.concatenate(
        [np.asarray(res.results[c]["out"]) for c in range(NCORES)], axis=0
    )
    return np.ascontiguousarray(out.reshape(B, C, H, W).astype(np.float32))


# revision 14
# speedup vs baseline: 1.0593x; 1.0593x over previous
"""Trainium2 Bass kernel for nn_AttentionBlock (GroupNorm + 4-head self-attention
+ projection + residual) on x:(16,512,32,32) fp32.

Sharding: data-parallel over batch across 8 NeuronCores (2 batch elements per
core), weights replicated, no collectives.

Per-core dataflow (channels-on-partitions layouts):
  GroupNorm: bn_stats per channel -> group reduce/broadcast via tiny selector
             matmuls -> xn = x*A + B (fp16)
  QKV: q,k in (o,s) layout, v computed transposed (s,o) by using xn as the
       stationary matmul operand.
  Attention per head, transposed layout, no max subtraction (|S| < ~8 so exp
       is safe): ST = k^T q -> E = exp(ST) (fp16, ScalarE) -> denominator
       broadcast = ones^T @ (pairwise-summed E) -> fast reciprocal (VectorE)
       -> out = (vT^T @ E) * recip
  Proj: bias + residual fused into one scalar_tensor_tensor (fp32, exact
       residual).

All matmuls are fp16 (11-bit mantissa; accumulation is always fp32 in PSUM).
This is deliberate: fp16 matmuls lower to LDWEIGHTS+MATMUL which together
carry two sync waits, while f32/f32r self-loading matmuls carry only ONE --
walrus rejects multi-source dependency patterns otherwise. All constants
arrive in a single fp16 DMA (f32 biases bit-packed into fp16 pairs), x in one
DMA, outputs in one DMA: 3 DMA queues + 3 engines keeps the kernel-tail drain
under the walrus sync-wait cap.
"""

import numpy as np

B, C, H, W = 16, 512, 32, 32
N = H * W  # 1024 spatial positions
HEADS = 4
HD = C // HEADS  # 128
EPS = 1e-5
NCORES = 8
BPC = B // NCORES  # 2 batches per core
CT = C // 128  # 4 channel tiles
NT = N // 512  # 2 free-dim tiles of 512

# fp16 constant-pack column offsets
_HOFF_WQKV = 0  # (128, CT*3C) = 6144
_HOFF_WPROJ = _HOFF_WQKV + CT * 3 * C  # (128, CT*C) = 2048
_HOFF_BV = _HOFF_WPROJ + CT * C  # row 0: (1, C)
_HOFF_ONES = _HOFF_BV + C  # (128, 128) of 1.0
_HOFF_SEL = _HOFF_ONES + 128  # (128, 8) group selector
_HOFF_SELT = _HOFF_SEL + 8  # rows 0:8: (8, 128)
_HOFF_F32 = _HOFF_SELT + 128  # f32 consts as fp16 hi+lo pairs: 20 f32 = 2x20
_PACK16_W = _HOFF_F32 + 40

_CACHE = {}


def build_program():
    import concourse.bass as bass
    import concourse.tile as tile
    from concourse import bacc, mybir

    f32 = mybir.dt.float32
    fp16 = mybir.dt.float16
    FT = mybir.ActivationFunctionType
    OP = mybir.AluOpType

    nc = bacc.Bacc(None, target_bir_lowering=False)

    x_d = nc.dram_tensor("x", [BPC, CT, 128, N], f32, kind="ExternalInput")
    pack_d = nc.dram_tensor("pack", [128, _PACK16_W], fp16, kind="ExternalInput")
    out_d = nc.dram_tensor("out", [BPC, CT, 128, N], f32, kind="ExternalOutput")

    ctx = tile.TileContext(nc)
    with ctx as tc, tc.tile_pool(name="consts", bufs=1) as consts:
        wp = consts.tile([128, _PACK16_W], fp16, tag="wp")
        nc.sync.dma_start(out=wp, in_=pack_d[:])
        wqkvT = wp[:, _HOFF_WQKV : _HOFF_WQKV + CT * 3 * C].rearrange(
            "p (t o) -> p t o", t=CT
        )
        wprojT = wp[:, _HOFF_WPROJ : _HOFF_WPROJ + CT * C].rearrange(
            "p (t o) -> p t o", t=CT
        )
        bvrow = wp[0:1, _HOFF_BV : _HOFF_BV + C]
        ones_h = wp[:, _HOFF_ONES : _HOFF_ONES + 128]
        ones1 = ones_h[0:1, :]
        sel = wp[:, _HOFF_SEL : _HOFF_SEL + 8]
        selT = wp[0:8, _HOFF_SELT : _HOFF_SELT + 128]
        # f32 bias constants travel as fp16 hi+lo pairs (bitcasting f32 into
        # fp16 lanes can produce NaN patterns the simulator rejects)
        fbias = consts.tile([128, 20], f32, tag="fbias")
        nc.vector.tensor_add(
            out=fbias,
            in0=wp[:, _HOFF_F32 : _HOFF_F32 + 20],
            in1=wp[:, _HOFF_F32 + 20 : _HOFF_F32 + 40],
        )
        bqk = fbias[:, 0:8]
        gb = fbias[:, 8:16].rearrange("p (t two) -> p t two", t=CT)
        bproj = fbias[:, 16:20]
        eps8 = consts.tile([8, 1], f32, tag="eps8")
        nc.vector.memset(eps8, EPS)

        with (
            tc.tile_pool(name="sb", bufs=1) as sb,
            tc.tile_pool(name="ps", bufs=1, space="PSUM") as ps,
        ):
            # x arrives per batch so batch 0's GroupNorm starts early and
            # batch 1's load overlaps batch 0 compute
            x_all = sb.tile([128, BPC, CT, N], f32, tag="x")
            for b in range(BPC):
                nc.sync.dma_start(
                    out=x_all[:, b], in_=x_d[b].rearrange("t p n -> p t n")
                )
            out_all = sb.tile([128, BPC, CT, N], f32, tag="outsb")

            for b in range(BPC):
                x_b = x_all[:, b]

                # ---------------- GroupNorm ----------------
                gst = sb.tile([8, CT, 3], f32, tag="gst", bufs=2)
                for t in range(CT):
                    stats6 = sb.tile([128, 2, 6], f32, tag="stats6", bufs=2)
                    xv = x_b[:, t, :].rearrange("p (a c) -> p a c", c=512)
                    for a in range(2):
                        nc.vector.bn_stats(out=stats6[:, a, :], in_=xv[:, a, :])
                    stats3 = sb.tile([128, 3], fp16, tag="stats3", bufs=2)
                    nc.vector.bn_aggr(out=stats3[:, 0:2], in_=stats6)
                    nc.vector.tensor_mul(
                        out=stats3[:, 2:3], in0=stats3[:, 0:1], in1=stats3[:, 0:1]
                    )
                    gsum_ps = ps.tile([8, 3], f32, tag="work", bufs=2)
                    nc.tensor.matmul(gsum_ps, sel, stats3, start=True, stop=True)
                    nc.vector.tensor_copy(out=gst[:, t, :], in_=gsum_ps)

                # group mean/var/rstd on (8, CT) views; rstd = exp(-0.5*ln(var+eps))
                rg = sb.tile([8, CT, 2], fp16, tag="rg", bufs=2)
                nc.vector.tensor_scalar(
                    out=rg[:, :, 1],
                    in0=gst[:, :, 0],
                    scalar1=1.0 / 16.0,
                    scalar2=None,
                    op0=OP.mult,
                )
                ex2 = sb.tile([8, CT], f32, tag="ex2", bufs=2)
                nc.vector.tensor_add(out=ex2, in0=gst[:, :, 1], in1=gst[:, :, 2])
                nc.vector.tensor_scalar(
                    out=ex2, in0=ex2, scalar1=1.0 / 16.0, scalar2=None, op0=OP.mult
                )
                msq = sb.tile([8, CT], f32, tag="msq", bufs=2)
                nc.vector.tensor_mul(out=msq, in0=rg[:, :, 1], in1=rg[:, :, 1])
                nc.vector.tensor_sub(out=ex2, in0=ex2, in1=msq)
                lnv = sb.tile([8, CT], f32, tag="lnv", bufs=2)
                nc.scalar.activation(out=lnv, in_=ex2, func=FT.Ln, bias=eps8)
                rr = sb.tile([8, CT], f32, tag="rr", bufs=2)
                nc.scalar.activation(out=rr, in_=lnv, func=FT.Exp, scale=-0.5)
                nc.vector.tensor_copy(out=rg[:, :, 0], in_=rr)

                ab = sb.tile([128, CT, 2], f32, tag="ab", bufs=2)
                for t in range(CT):
                    cb_ps = ps.tile([128, 2], f32, tag="work", bufs=2)
                    nc.tensor.matmul(cb_ps, selT, rg[:, t, :], start=True, stop=True)
                    nc.vector.tensor_mul(
                        out=ab[:, t, 0:1], in0=cb_ps[:, 0:1], in1=gb[:, t, 0:1]
                    )
                    tmp1 = sb.tile([128, 1], f32, tag="tmp1", bufs=2)
                    nc.vector.tensor_mul(out=tmp1, in0=cb_ps[:, 1:2], in1=ab[:, t, 0:1])
                    nc.vector.tensor_sub(out=ab[:, t, 1:2], in0=gb[:, t, 1:2], in1=tmp1)

                xn = sb.tile([128, CT, N], fp16, tag="xn", bufs=1)
                for t in range(CT):
                    nc.vector.tensor_scalar(
                        out=xn[:, t, :],
                        in0=x_b[:, t, :],
                        scalar1=ab[:, t, 0:1],
                        scalar2=ab[:, t, 1:2],
                        op0=OP.mult,
                        op1=OP.add,
                    )

                # ---------------- QKV ----------------
                qk = sb.tile([128, 8, N], fp16, tag="qk", bufs=1)
                for m in range(8):
                    qkps = ps.tile([128, N], f32, tag="work", bufs=2)
                    for kk in range(CT):
                        lhs = wqkvT[:, kk, m * 128 : (m + 1) * 128]
                        for n in range(NT):
                            nc.tensor.matmul(
                                qkps[:, n * 512 : (n + 1) * 512],
                                lhs,
                                xn[:, kk, n * 512 : (n + 1) * 512],
                                start=(kk == 0),
                                stop=(kk == CT - 1),
                            )
                    nc.vector.tensor_scalar(
                        out=qk[:, m, :],
                        in0=qkps,
                        scalar1=bqk[:, m : m + 1],
                        scalar2=None,
                        op0=OP.add,
                    )

                vT = sb.tile([128, 8, C], fp16, tag="vT", bufs=1)
                for sm in range(8):
                    vps = ps.tile([128, C], f32, tag="work", bufs=2)
                    nc.tensor.matmul(vps, ones1, bvrow, start=True, stop=False)
                    for kk in range(CT):
                        nc.tensor.matmul(
                            vps,
                            xn[:, kk, sm * 128 : (sm + 1) * 128],
                            wqkvT[:, kk, 2 * C : 3 * C],
                            start=False,
                            stop=(kk == CT - 1),
                        )
                    nc.vector.tensor_copy(out=vT[:, sm, :], in_=vps)

                # ---------------- attention ----------------
                attn = sb.tile([128, CT, N], fp16, tag="attn", bufs=1)
                for h in range(HEADS):
                    q_h = qk[:, h, :]
                    k_h = qk[:, 4 + h, :]
                    av_ps = ps.tile([128, N], f32, tag="avdn", bufs=2)
                    dn_ps = ps.tile([128, N], f32, tag="avdn", bufs=2)
                    prev_e = None
                    for mt in range(8):
                        st_ps = ps.tile([128, N], f32, tag="work", bufs=2)
                        lhs_k = k_h[:, mt * 128 : (mt + 1) * 128]
                        for n in range(NT):
                            nc.tensor.matmul(
                                st_ps[:, n * 512 : (n + 1) * 512],
                                lhs_k,
                                q_h[:, n * 512 : (n + 1) * 512],
                                start=True,
                                stop=True,
                            )
                        e_t = sb.tile([128, N], fp16, tag="E", bufs=8)
                        nc.scalar.activation(out=e_t, in_=st_ps, func=FT.Exp)
                        lhs_v = vT[:, mt, h * 128 : (h + 1) * 128]
                        for n in range(NT):
                            nc.tensor.matmul(
                                av_ps[:, n * 512 : (n + 1) * 512],
                                lhs_v,
                                e_t[:, n * 512 : (n + 1) * 512],
                                start=(mt == 0),
                                stop=(mt == 7),
                            )
                        if mt % 2 == 0:
                            prev_e = e_t
                        else:
                            j = mt // 2
                            pr = sb.tile([128, N], fp16, tag="pair", bufs=3)
                            nc.vector.tensor_add(out=pr, in0=prev_e, in1=e_t)
                            for n in range(NT):
                                nc.tensor.matmul(
                                    dn_ps[:, n * 512 : (n + 1) * 512],
                                    ones_h,
                                    pr[:, n * 512 : (n + 1) * 512],
                                    start=(j == 0),
                                    stop=(j == 3),
                                )
                    rec = sb.tile([128, N], f32, tag="rec", bufs=2)
                    nc.vector.reciprocal_approx_fast(out=rec, in_=dn_ps)
                    nc.vector.tensor_mul(out=attn[:, h, :], in0=av_ps, in1=rec)

                # ---------------- proj + bias + residual ----------------
                for m in range(CT):
                    prps = ps.tile([128, N], f32, tag="work", bufs=2)
                    for t in range(CT):
                        lhs = wprojT[:, t, m * 128 : (m + 1) * 128]
                        for n in range(NT):
                            nc.tensor.matmul(
                                prps[:, n * 512 : (n + 1) * 512],
                                lhs,
                                attn[:, t, n * 512 : (n + 1) * 512],
                                start=(t == 0),
                                stop=(t == CT - 1),
                            )
                    nc.vector.scalar_tensor_tensor(
                        out=out_all[:, b, m, :],
                        in0=prps,
                        scalar=bproj[:, m : m + 1],
                        in1=x_b[:, m, :],
                        op0=OP.add,
                        op1=OP.add,
                    )
                nc.sync.dma_start(
                    out=out_d[b].rearrange("t p n -> p t n"), in_=out_all[:, b]
                )

    nc.finalize()
    return nc


def _host_prep(x, gamma, beta, w_qkv, b_qkv, w_proj, b_proj):
    scale = float(HD) ** -0.5
    x = np.asarray(x, np.float32)
    gamma = np.asarray(gamma, np.float32)
    beta = np.asarray(beta, np.float32)
    w_qkv = np.asarray(w_qkv, np.float32)
    b_qkv = np.asarray(b_qkv, np.float32)
    w_proj = np.asarray(w_proj, np.float32)
    b_proj = np.asarray(b_proj, np.float32)

    wq = w_qkv.copy()
    wq[:C] *= scale  # fold the attention scale into the q weights/bias
    bq = b_qkv.copy()
    bq[:C] *= scale

    pack = np.zeros((128, _PACK16_W), np.float16)
    pack[:, _HOFF_WQKV : _HOFF_WQKV + CT * 3 * C] = (
        np.ascontiguousarray(wq.T)
        .reshape(CT, 128, 3 * C)
        .transpose(1, 0, 2)
        .reshape(128, -1)
        .astype(np.float16)
    )
    pack[:, _HOFF_WPROJ : _HOFF_WPROJ + CT * C] = (
        np.ascontiguousarray(w_proj.T)
        .reshape(CT, 128, C)
        .transpose(1, 0, 2)
        .reshape(128, -1)
        .astype(np.float16)
    )
    pack[0, _HOFF_BV : _HOFF_BV + C] = bq[2 * C :].astype(np.float16)
    pack[:, _HOFF_ONES : _HOFF_ONES + 128] = np.float16(1.0)
    selm = np.zeros((128, 8), np.float16)
    selm[np.arange(128), np.arange(128) // 16] = np.float16(1.0)
    pack[:, _HOFF_SEL : _HOFF_SEL + 8] = selm
    pack[0:8, _HOFF_SELT : _HOFF_SELT + 128] = selm.T

    fpart = np.zeros((128, 20), np.float32)
    fpart[:, 0:8] = bq[: 2 * C].reshape(8, 128).T
    fpart[:, 8:16] = (
        np.stack([gamma.reshape(CT, 128), beta.reshape(CT, 128)], axis=-1)
        .transpose(1, 0, 2)
        .reshape(128, -1)
    )
    fpart[:, 16:20] = b_proj.reshape(CT, 128).T
    hi = fpart.astype(np.float16)
    lo = (fpart - hi.astype(np.float32)).astype(np.float16)
    pack[:, _HOFF_F32 : _HOFF_F32 + 20] = hi
    pack[:, _HOFF_F32 + 20 : _HOFF_F32 + 40] = lo

    x_r = np.ascontiguousarray(x.reshape(B, CT, 128, N))
    in_maps = [
        {
            "pack": pack,
            "x": np.ascontiguousarray(
                x_r[c * BPC : (c + 1) * BPC].reshape(BPC, CT, 128, N)
            ),
        }
        for c in range(NCORES)
    ]
    return in_maps


def kernel(x, gamma, beta, w_qkv, b_qkv, w_proj, b_proj, _trace=False):
    from concourse.bass_utils import run_bass_kernel_spmd

    if "nc" not in _CACHE:
        _CACHE["nc"] = build_program()
    nc = _CACHE["nc"]

    in_maps = _host_prep(x, gamma, beta, w_qkv, b_qkv, w_proj, b_proj)
    res = run_bass_kernel_spmd(
        nc, in_maps, core_ids=list(range(NCORES)), trace=_trace
    )
    _CACHE["last_result"] = res
    out = np.concatenate(
        [np.asarray(res.results[c]["out"]) for c in range(NCORES)], axis=0
    )
    return np.ascontiguousarray(out.reshape(B, C, H, W).astype(np.float32))
